# revision 5
# baseline (speedup 1.0000x reference)
"""Trainium2 Bass kernel: accuracy evaluator (argmax == label, mean).

reference: idx = argmax(prediction[M,K,N,B,C], axis=-1)
           out = mean(idx == label) over M,K,N,B  (scalar f32)

Strategy (8 NeuronCores, data parallel over M).  Each core streams its
pred shard [2,16,16,2048,10] -> [S=512 slices, B*C=20480] f32 through
SBUF in quarter-tiles [128, 5120] at full DMA BW (~103us/pass, the
memory roofline).

Key restructure vs the old 2-tree kernel: the batch is PERMUTED on the
host so labels are sorted (the mean over b is permutation-invariant, so
this is pure layout).  Within a chunk the label-class values then live
on a few constant-stride segments view3[:, bs:be, k], so DVE reads
x = pred[b, label_b] directly -- no gather, no negmask, no second tree:

    rmax = reduce_max(view3 [P, nb, 10], axis=X)       (1 op)
    scr[bs:be] = is_ge(view3[:, bs:be, k], rmax[bs:be])  (~3 runs/chunk)
    cnt2d += scr

Mode-"f" chunks do this in f32 (exact ties-as-correct semantics: zero
error on randn data).  Mode-"b" chunks let Act convert the tile to bf16
first and DVE reduce/compare in bf16 (2x DVE rate, adds ~1e-3 bf16-tie
overcount) -- used to keep DVE safely under the DMA stream.

Chunks: 14 full quarters [128,5120] + last 2 quarters as 8 minis
[128,1280] to shorten the pipeline drain after the final DMA.
The label-derived run structure is baked into the program's access
patterns; the Bass program is cached per (reps, label-hash).
Host sums the per-partition correct counts across cores.
"""

import os
import sys
from contextlib import ExitStack

import numpy as np

for _p in ("/opt/trn_rl_repo", os.path.expanduser("~/.axon_site/_ro/trn_rl_repo")):
    if os.path.isdir(_p) and _p not in sys.path:
        sys.path.insert(0, _p)

import concourse.bass as bass
from concourse import mybir
from concourse.bass_utils import run_bass_kernel_spmd

M, K, N, B, C = 16, 16, 16, 2048, 10
NCORES = 8
P = 128                       # SBUF partitions
S = (M // NCORES) * K * N     # 512 slices per core
NT = S // P                   # 4 s-tiles
NQ = 4                        # quarters per s-tile
BQ = B // NQ                  # 512 b-rows per full chunk
F = BQ * C                    # 5120 free elements per full chunk
FS = B * C                    # 20480 free elements per s-tile row
MINI = F // 4                 # 1280 cols per mini chunk
MB = MINI // C                # 128 b-rows per mini

NBUF = 6      # pred DMA slots (quarter-sized)
NCONV = 3     # bf16 conv buffers (mode-"b" chunks)

# chunk list: (ti, col0, ncols, mode); mode "f": DVE reduces/compares
# the f32 ptile directly; mode "b": Act converts to bf16 first.
MODE_B = frozenset((1, 3, 5, 7, 9, 11))
CHUNKS = []
for _t in range(NT * NQ - 2):
    _ti, _q = divmod(_t, NQ)
    CHUNKS.append((_ti, _q * F, F, "b" if _t in MODE_B else "f"))
for _m in range(8):
    CHUNKS.append((3, 2 * F + _m * MINI, MINI, "f"))
NCHUNKS = len(CHUNKS)         # 22

_cache: dict = {}


def _chunk_runs(lab_sorted):
    """Per chunk: list of (class k, bs, be) runs in within-chunk b coords."""
    bounds = np.searchsorted(lab_sorted, np.arange(C + 1))
    out = []
    for ti, col0, ncols, _mm in CHUNKS:
        b0 = col0 // C
        nb = ncols // C
        runs = []
        for k in range(C):
            bs = max(int(bounds[k]), b0)
            be = min(int(bounds[k + 1]), b0 + nb)
            if bs < be:
                runs.append((k, bs - b0, be - b0))
        out.append(runs)
    return out


def _build_nc(runs_per_chunk, reps: int = 1):
    f32 = mybir.dt.float32
    bf16 = mybir.dt.bfloat16
    # detect_race_conditions=False: the CoreSim race detector does not credit
    # same-engine program order (engines are in-order on HW), so raw-bass
    # same-engine chains trip it spuriously. Values are still checked.
    nc = bass.Bass(
        "TRN2",
        target_bir_lowering=False,
        debug=False,
        num_devices=NCORES,
        detect_race_conditions=False,
    )
    pred = nc.dram_tensor("pred", [S, FS], f32, kind="ExternalInput").ap()
    cnt = nc.dram_tensor("cnt", [P, 1], f32, kind="ExternalOutput").ap()

    niter = NCHUNKS * reps
    modes = [CHUNKS[g % NCHUNKS][3] for g in range(niter)]
    any_b = any(m == "b" for m in modes)

    with ExitStack() as ctx:
        ptiles = [
            ctx.enter_context(nc.sbuf_tensor(f"ptile{s}", [P, F], f32))
            for s in range(NBUF)
        ]
        rmaxf = [
            ctx.enter_context(nc.sbuf_tensor(f"rmaxf{i}", [P, BQ], f32))
            for i in range(2)
        ]
        scr = ctx.enter_context(nc.sbuf_tensor("scr", [P, BQ], f32))
        cnt2d = ctx.enter_context(nc.sbuf_tensor("cnt2d", [P, BQ], f32))
        cnt1 = ctx.enter_context(nc.sbuf_tensor("cnt1", [P, 1], f32))
        if any_b:
            convs = [
                ctx.enter_context(nc.sbuf_tensor(f"conv{b}", [P, F], bf16))
                for b in range(NCONV)
            ]
            rmaxb = [
                ctx.enter_context(nc.sbuf_tensor(f"rmaxb{i}", [P, BQ], bf16))
                for i in range(2)
            ]
            scrb = ctx.enter_context(nc.sbuf_tensor("scrb", [P, BQ], bf16))

        ld = [ctx.enter_context(nc.semaphore(f"ld{s}")) for s in range(NBUF)]
        sc = [ctx.enter_context(nc.semaphore(f"sc{s}")) for s in range(NBUF)]
        vd = ctx.enter_context(nc.semaphore("vd"))        # DVE finished
        st = ctx.enter_context(nc.semaphore("st"))        # result stored
        if any_b:
            cvr = ctx.enter_context(nc.semaphore("cvr"))  # conv consumed

        block = ctx.enter_context(nc.Block())

        @block.sync
        def _(sync):
            for g in range(niter):
                s = g % NBUF
                j = g // NBUF
                if g >= NBUF:
                    # slot s free once its single reader (DVE or Act) ran
                    sync.wait_ge(sc[s], j)
                ti, col0, ncols, _mm = CHUNKS[g % NCHUNKS]
                sync.dma_start(
                    ptiles[s][:, 0:ncols],
                    pred[ti * P : (ti + 1) * P, col0 : col0 + ncols],
                ).then_inc(ld[s], 16)
            sync.wait_ge(st, 16)

        @block.scalar
        def _(scalar):
            nbc = 0  # "b" chunks processed
            for g in range(niter):
                if modes[g] != "b":
                    continue
                s = g % NBUF
                j = g // NBUF
                ti, col0, ncols, _mm = CHUNKS[g % NCHUNKS]
                scalar.wait_ge(ld[s], 16 * (j + 1))
                if nbc >= NCONV:
                    scalar.wait_ge(cvr, nbc - NCONV + 1)
                nc.scalar.activation(
                    convs[nbc % NCONV][:, 0:ncols],
                    ptiles[s][:, 0:ncols],
                    mybir.ActivationFunctionType.Copy,
                ).then_inc(sc[s], 1)
                nbc += 1
            scalar.wait_ge(vd, 1)
            scalar.dma_start(cnt[:, :], cnt1[:, :]).then_inc(st, 16)

        add = mybir.AluOpType.add
        ge = mybir.AluOpType.is_ge
        mx = mybir.AluOpType.max

        @block.vector
        def _(vector):
            nc.vector.memset(cnt2d[:, :], 0.0)
            nbc = 0
            for g in range(niter):
                s = g % NBUF
                j = g // NBUF
                i = g % NCHUNKS
                ti, col0, ncols, mm = CHUNKS[i]
                nb = ncols // C
                runs = runs_per_chunk[i]
                if mm == "f":
                    vector.wait_ge(ld[s], 16 * (j + 1))
                    view3 = ptiles[s][:, 0:ncols].rearrange(
                        "p (f c) -> p f c", c=C
                    )
                    rm = rmaxf[g % 2]
                    sv = scr
                else:
                    vector.wait_ge(sc[s], j + 1)
                    view3 = convs[nbc % NCONV][:, 0:ncols].rearrange(
                        "p (f c) -> p f c", c=C
                    )
                    rm = rmaxb[g % 2]
                    sv = scrb
                    nbc += 1
                nc.vector.tensor_reduce(
                    rm[:, 0:nb], view3, axis=mybir.AxisListType.X, op=mx
                )
                for ri, (k, bs, be) in enumerate(runs):
                    inst = nc.vector.tensor_tensor(
                        sv[:, bs:be],
                        view3[:, bs:be, k : k + 1],
                        rm[:, bs:be],
                        op=ge,
                    )
                    if ri == len(runs) - 1:
                        # last read of the tile: frees the slot (f) or the
                        # conv buffer (b)
                        inst.then_inc(sc[s] if mm == "f" else cvr, 1)
                nc.vector.tensor_tensor(
                    cnt2d[:, 0:nb], cnt2d[:, 0:nb], sv[:, 0:nb], op=add
                )
            nc.vector.reduce_sum(
                cnt1[:, :], cnt2d[:, :], axis=mybir.AxisListType.X
            ).then_inc(vd, 1)

    return nc


def _get_nc(lab_sorted, reps: int = 1):
    key = ("nc", reps, lab_sorted.tobytes())
    if key not in _cache:
        _cache[key] = _build_nc(_chunk_runs(lab_sorted), reps)
    return _cache[key]


def _host_inputs(prediction, label):
    pred = np.asarray(prediction, dtype=np.float32).reshape(
        NCORES, S, B, C
    )
    lab = np.asarray(label).astype(np.int64).reshape(B)
    perm = np.argsort(lab, kind="stable")
    lab_sorted = lab[perm]
    shards = pred[:, :, perm, :].reshape(NCORES, S, FS)
    in_maps = [
        {"pred": np.ascontiguousarray(shards[k])} for k in range(NCORES)
    ]
    return in_maps, lab_sorted


def run(prediction, label, reps: int = 1, **spmd_kwargs):
    """Run on HW; returns (scalar_output, BassKernelResults)."""
    in_maps, lab_sorted = _host_inputs(prediction, label)
    nc = _get_nc(lab_sorted, reps)
    res = run_bass_kernel_spmd(nc, in_maps, list(range(NCORES)), **spmd_kwargs)
    total = 0.0
    for r in res.results:
        total += float(np.asarray(r["cnt"], dtype=np.float64).sum())
    out = np.float32(total / float(reps * M * K * N * B))
    return out, res


def kernel(prediction, label):
    out, _ = run(prediction, label)
    return out


# revision 11
# speedup vs baseline: 1.7871x; 1.7871x over previous
"""Trainium2 Bass kernel: accuracy evaluator (argmax == label, mean).

reference: idx = argmax(prediction[M,K,N,B,C], axis=-1)
           out = mean(idx == label) over M,K,N,B  (scalar f32)

Strategy (8 NeuronCores, data parallel over M).  Each core streams its
pred shard [2,16,16,2048,10] -> [S=512 slices, B*C=20480] f32 through
SBUF in quarter-tiles [128, 5120] at full DMA BW.  The shard is
host-retiled so consecutive chunks are fully HBM-contiguous 2.5 MB
reads (the whole 40 MiB/core pass is one sequential HBM sweep,
measured 445-540 GB/s/core; the DMA floor is ~74-95us depending on
HBM co-tenancy, and the kernel tracks it).

Key restructure vs the old 2-tree kernel: the batch is PERMUTED on the
host so labels are sorted (the mean over b is permutation-invariant, so
this is pure layout).  Within a chunk the label-class values then live
on a few constant-stride segments view3[:, bs:be, k], so DVE reads
x = pred[b, label_b] directly -- no gather, no negmask, no second tree:

    rmax = reduce_max(view3 [P, nb, 10], axis=X)       (1 op)
    scr[bs:be] = is_ge(view3[:, bs:be, k], rmax[bs:be])  (~3 runs/chunk)
    cnt2d += scr

Mode-"f" chunks do this in f32 (exact ties-as-correct semantics: zero
error on randn data).  Mode-"b" chunks let Act convert the tile to bf16
first and DVE reduce/compare in bf16 (2x DVE rate, adds ~1e-3 bf16-tie
overcount) -- used to keep DVE safely under the DMA stream.

Chunks: 14 full quarters [128,5120] + last 2 quarters as 8 minis
[128,1280] to shorten the pipeline drain after the final DMA.
The label-derived run structure is baked into the program's access
patterns; the Bass program is cached per (reps, label-hash).
Host sums the per-partition correct counts across cores.
"""

import os
import sys
from contextlib import ExitStack

import numpy as np

for _p in ("/opt/trn_rl_repo", os.path.expanduser("~/.axon_site/_ro/trn_rl_repo")):
    if os.path.isdir(_p) and _p not in sys.path:
        sys.path.insert(0, _p)

import concourse.bass as bass
from concourse import mybir
from concourse.bass_utils import run_bass_kernel_spmd

M, K, N, B, C = 16, 16, 16, 2048, 10
NCORES = 8
P = 128                       # SBUF partitions
S = (M // NCORES) * K * N     # 512 slices per core
NT = S // P                   # 4 s-tiles
NQ = 4                        # quarters per s-tile
BQ = B // NQ                  # 512 b-rows per full chunk
F = BQ * C                    # 5120 free elements per full chunk
FS = B * C                    # 20480 free elements per s-tile row
MINI = F // 4                 # 1280 cols per mini chunk
MB = MINI // C                # 128 b-rows per mini

NBUF = int(os.environ.get("K_NBUF", "6"))   # pred DMA slots
NCONV = 3     # bf16 conv buffers (mode-"b" chunks)

# chunk list: (ti, col0, ncols, mode); mode "f": DVE reduces/compares
# the f32 ptile directly; mode "b": Act converts to bf16 first.
# K_NB full quarters (spread evenly) go through the Act bf16 path.
_NB = int(os.environ.get("K_NB", "10"))
MODE_B = frozenset(range(14)) if _NB >= 14 else frozenset(
    round(i * 14 / max(_NB, 1)) for i in range(_NB)
)
CHUNKS = []
for _t in range(NT * NQ - 2):
    _ti, _q = divmod(_t, NQ)
    CHUNKS.append((_ti, _q * F, F, "b" if _t in MODE_B else "f"))
for _m in range(8):
    CHUNKS.append((3, 2 * F + _m * MINI, MINI, "f"))
NCHUNKS = len(CHUNKS)         # 22

_cache: dict = {}


def _chunk_runs(lab_sorted):
    """Per chunk: list of (class k, bs, be) runs in within-chunk b coords."""
    bounds = np.searchsorted(lab_sorted, np.arange(C + 1))
    out = []
    for ti, col0, ncols, _mm in CHUNKS:
        b0 = col0 // C
        nb = ncols // C
        runs = []
        for k in range(C):
            bs = max(int(bounds[k]), b0)
            be = min(int(bounds[k + 1]), b0 + nb)
            if bs < be:
                runs.append((k, bs - b0, be - b0))
        out.append(runs)
    return out


def _build_nc(runs_per_chunk, reps: int = 1):
    f32 = mybir.dt.float32
    bf16 = mybir.dt.bfloat16
    # detect_race_conditions=False: the CoreSim race detector does not credit
    # same-engine program order (engines are in-order on HW), so raw-bass
    # same-engine chains trip it spuriously. Values are still checked.
    nc = bass.Bass(
        "TRN2",
        target_bir_lowering=False,
        debug=False,
        num_devices=NCORES,
        detect_race_conditions=False,
    )
    pred = nc.dram_tensor("pred", [S * NQ, F], f32, kind="ExternalInput").ap()
    cnt = nc.dram_tensor("cnt", [P, 1], f32, kind="ExternalOutput").ap()

    niter = NCHUNKS * reps
    modes = [CHUNKS[g % NCHUNKS][3] for g in range(niter)]
    any_b = any(m == "b" for m in modes)

    with ExitStack() as ctx:
        ptiles = [
            ctx.enter_context(nc.sbuf_tensor(f"ptile{s}", [P, F], f32))
            for s in range(NBUF)
        ]
        rmaxf = [
            ctx.enter_context(nc.sbuf_tensor(f"rmaxf{i}", [P, BQ], f32))
            for i in range(2)
        ]
        scr = ctx.enter_context(nc.sbuf_tensor("scr", [P, BQ], f32))
        cnt2d = ctx.enter_context(nc.sbuf_tensor("cnt2d", [P, BQ], f32))
        cnt1 = ctx.enter_context(nc.sbuf_tensor("cnt1", [P, 1], f32))
        if any_b:
            convs = [
                ctx.enter_context(nc.sbuf_tensor(f"conv{b}", [P, F], bf16))
                for b in range(NCONV)
            ]
            rmaxb = [
                ctx.enter_context(nc.sbuf_tensor(f"rmaxb{i}", [P, BQ], bf16))
                for i in range(2)
            ]
            scrb = ctx.enter_context(nc.sbuf_tensor("scrb", [P, BQ], bf16))

        ld = [ctx.enter_context(nc.semaphore(f"ld{s}")) for s in range(NBUF)]
        sc = [ctx.enter_context(nc.semaphore(f"sc{s}")) for s in range(NBUF)]
        vd = ctx.enter_context(nc.semaphore("vd"))        # DVE finished
        st = ctx.enter_context(nc.semaphore("st"))        # result stored
        if any_b:
            cvr = ctx.enter_context(nc.semaphore("cvr"))  # conv consumed

        block = ctx.enter_context(nc.Block())

        def issue_dma(eng, g):
            s = g % NBUF
            j = g // NBUF
            if g >= NBUF:
                # slot s free once its single reader (DVE or Act) ran
                eng.wait_ge(sc[s], j)
            ti, col0, ncols, _mm = CHUNKS[g % NCHUNKS]
            row0 = (ti * NQ + col0 // F) * P
            cb = col0 % F
            eng.dma_start(
                ptiles[s][:, 0:ncols],
                pred[row0 : row0 + P, cb : cb + ncols],
            ).then_inc(ld[s], 16)

        @block.sync
        def _(sync):
            # single sync-engine HWDGE queue; a second stream on the
            # Pool (SWDGE) queue measured far worse -- don't split.
            for g in range(niter):
                issue_dma(sync, g)
            sync.wait_ge(st, 16)

        @block.scalar
        def _(scalar):
            nbc = 0  # "b" chunks processed
            for g in range(niter):
                if modes[g] != "b":
                    continue
                s = g % NBUF
                j = g // NBUF
                ti, col0, ncols, _mm = CHUNKS[g % NCHUNKS]
                scalar.wait_ge(ld[s], 16 * (j + 1))
                if nbc >= NCONV:
                    scalar.wait_ge(cvr, nbc - NCONV + 1)
                nc.scalar.activation(
                    convs[nbc % NCONV][:, 0:ncols],
                    ptiles[s][:, 0:ncols],
                    mybir.ActivationFunctionType.Copy,
                ).then_inc(sc[s], 1)
                nbc += 1
            scalar.wait_ge(vd, 1)
            scalar.dma_start(cnt[:, :], cnt1[:, :]).then_inc(st, 16)

        add = mybir.AluOpType.add
        ge = mybir.AluOpType.is_ge
        mx = mybir.AluOpType.max

        @block.vector
        def _(vector):
            nc.vector.memset(cnt2d[:, :], 0.0)
            nbc = 0
            for g in range(niter):
                s = g % NBUF
                j = g // NBUF
                i = g % NCHUNKS
                ti, col0, ncols, mm = CHUNKS[i]
                nb = ncols // C
                runs = runs_per_chunk[i]
                if mm == "f":
                    vector.wait_ge(ld[s], 16 * (j + 1))
                    view3 = ptiles[s][:, 0:ncols].rearrange(
                        "p (f c) -> p f c", c=C
                    )
                    rm = rmaxf[g % 2]
                    sv = scr
                else:
                    vector.wait_ge(sc[s], j + 1)
                    view3 = convs[nbc % NCONV][:, 0:ncols].rearrange(
                        "p (f c) -> p f c", c=C
                    )
                    rm = rmaxb[g % 2]
                    sv = scrb
                    nbc += 1
                nc.vector.tensor_reduce(
                    rm[:, 0:nb], view3, axis=mybir.AxisListType.X, op=mx
                )
                for ri, (k, bs, be) in enumerate(runs):
                    inst = nc.vector.tensor_tensor(
                        sv[:, bs:be],
                        view3[:, bs:be, k : k + 1],
                        rm[:, bs:be],
                        op=ge,
                    )
                    if ri == len(runs) - 1:
                        # last read of the tile: frees the slot (f) or the
                        # conv buffer (b)
                        inst.then_inc(sc[s] if mm == "f" else cvr, 1)
                nc.vector.tensor_tensor(
                    cnt2d[:, 0:nb], cnt2d[:, 0:nb], sv[:, 0:nb], op=add
                )
            nc.vector.reduce_sum(
                cnt1[:, :], cnt2d[:, :], axis=mybir.AxisListType.X
            ).then_inc(vd, 1)

    return nc


def _get_nc(lab_sorted, reps: int = 1):
    key = ("nc", reps, lab_sorted.tobytes())
    if key not in _cache:
        _cache[key] = _build_nc(_chunk_runs(lab_sorted), reps)
    return _cache[key]


def _host_inputs(prediction, label):
    pred = np.asarray(prediction, dtype=np.float32).reshape(
        NCORES, S, B, C
    )
    lab = np.asarray(label).astype(np.int64).reshape(B)
    perm = np.argsort(lab, kind="stable")
    lab_sorted = lab[perm]
    # tiled layout: [tile, quarter, partition, F] so each full-quarter
    # DMA is one fully HBM-contiguous 2.5 MB read
    shards = (
        pred[:, :, perm, :]
        .reshape(NCORES, NT, P, NQ, F)
        .transpose(0, 1, 3, 2, 4)
        .reshape(NCORES, S * NQ, F)
    )
    in_maps = [
        {"pred": np.ascontiguousarray(shards[k])} for k in range(NCORES)
    ]
    return in_maps, lab_sorted


def run(prediction, label, reps: int = 1, **spmd_kwargs):
    """Run on HW; returns (scalar_output, BassKernelResults)."""
    in_maps, lab_sorted = _host_inputs(prediction, label)
    nc = _get_nc(lab_sorted, reps)
    res = run_bass_kernel_spmd(nc, in_maps, list(range(NCORES)), **spmd_kwargs)
    total = 0.0
    for r in res.results:
        total += float(np.asarray(r["cnt"], dtype=np.float64).sum())
    out = np.float32(total / float(reps * M * K * N * B))
    return out, res


def kernel(prediction, label):
    out, _ = run(prediction, label)
    return out
